# revision 1
# baseline (speedup 1.0000x reference)
"""LocallyConnected2d Trainium2 kernel.

y[b,o,h,w] = sum_{i,ky,kx} x[b,i,h+ky-1,w+kx-1] * weight[i,o,h,w,ky,kx] + bias[o,h,w]

Shapes: x [64,64,32,32], weight [64,64,32,32,3,3], bias [64,32,32] -> y [64,64,32,32].

Strategy
--------
Spatial sharding over H_out: 8 cores x 4 output rows each (receptive fields
need rows h-1..h+4 of x, packed per-core on host).

Per output location (h,w): a K=576 x M=64(cout) x N=64(batch) matmul,
executed as 5 PSUM-accumulating matmuls: 4 chunks of K=128 (each chunk = two
kernel offsets x 64 cin) + 1 tail chunk of K=64 (offset (2,2)).

A K=128 chunk spans two kernel offsets whose x data must appear at the SAME
free-dim offset on partitions 0-63 and 64-127. We pre-shift the bottom copy on
host: X1 has the bottom half shifted by 1 (serves pairs (ky,0)+(ky,1)), X34 is
shifted by 34 (serves pair (0,2)+(1,2)). Offsets are in units of 64-batch
blocks over the flattened (row, col) slab of the padded x slice.

All inputs are host-packed into exact per-core SBUF images so every DMA is a
plain contiguous [P, F] load.
"""

import sys

sys.path.insert(0, "/opt/trn_rl_repo")

import numpy as np

B, CIN, COUT, H, W = 64, 64, 64, 32, 32
K = 3
HOUT, WOUT = 32, 32
NCORES = 8
ROWS = HOUT // NCORES  # output rows per core
SLAB_R = ROWS + 2      # x rows needed per core (halo)
SLAB_C = W + 2         # padded width
RC = SLAB_R * SLAB_C   # flattened (row, col) length

# chunk pairing: j=0..3 -> (ky0,kx0)+(ky1,kx1); tail = (2,2)
PAIRS = [((0, 0), (0, 1)), ((1, 0), (1, 1)), ((2, 0), (2, 1)), ((0, 2), (1, 2))]
TAIL = (2, 2)

_nc_cache = {}


def _build_bass():
    import concourse.bass as bass
    import concourse.tile as tile
    from concourse import bacc, mybir

    f32 = mybir.dt.float32
    nc = bacc.Bacc(None, target_bir_lowering=False)

    x1_d = nc.dram_tensor("x1", (128, RC, B), f32, kind="ExternalInput")
    x34_d = nc.dram_tensor("x34", (128, RC, B), f32, kind="ExternalInput")
    wmain_d = nc.dram_tensor("wmain", (ROWS, 128, WOUT, 4, COUT), f32, kind="ExternalInput")
    wtail_d = nc.dram_tensor("wtail", (ROWS, 64, WOUT, COUT), f32, kind="ExternalInput")
    bias_d = nc.dram_tensor("bias", (ROWS, COUT, WOUT), f32, kind="ExternalInput")
    out_d = nc.dram_tensor("out", (ROWS, COUT, WOUT, B), f32, kind="ExternalOutput")

    with tile.TileContext(nc) as tc:
        with (
            tc.tile_pool(name="xpool", bufs=1) as xpool,
            tc.tile_pool(name="wpool", bufs=2) as wpool,
            tc.tile_pool(name="opool", bufs=2) as opool,
            tc.tile_pool(name="bpool", bufs=1) as bpool,
            tc.tile_pool(name="psum", bufs=8, space=bass.MemorySpace.PSUM) as psum,
        ):
            x1 = xpool.tile([128, RC, B], f32, tag="x1")
            x34 = xpool.tile([128, RC, B], f32, tag="x34")
            nc.sync.dma_start(x1[:], x1_d[:])
            nc.sync.dma_start(x34[:], x34_d[:])

            bi = bpool.tile([COUT, ROWS, WOUT], f32, tag="bias")
            nc.sync.dma_start(
                bi[:], bias_d.rearrange("h o w -> o h w")
            )

            for h in range(ROWS):
                wm = wpool.tile([128, WOUT, 4, COUT], f32, tag="wm")
                wt = wpool.tile([64, WOUT, COUT], f32, tag="wt")
                nc.sync.dma_start(wm[:], wmain_d[h])
                nc.sync.dma_start(wt[:], wtail_d[h])
                ot = opool.tile([COUT, WOUT, B], f32, tag="out")

                for w in range(WOUT):
                    ps = psum.tile([COUT, B], f32, tag="ps")
                    for j, ((ky0, kx0), _) in enumerate(PAIRS):
                        xsrc = x34 if j == 3 else x1
                        rc = (h + ky0) * SLAB_C + (w + kx0)
                        nc.tensor.matmul(
                            ps[:],
                            wm[:, w, j, :],
                            xsrc[:, rc, :],
                            start=(j == 0),
                            stop=False,
                        )
                    rc_t = (h + TAIL[0]) * SLAB_C + (w + TAIL[1])
                    nc.tensor.matmul(
                        ps[:],
                        wt[:, w, :],
                        x1[0:64, rc_t, :],
                        start=False,
                        stop=True,
                    )
                    nc.any.tensor_scalar_add(ot[:, w, :], ps[:], bi[:, h, w : w + 1])

                nc.sync.dma_start(out_d[h], ot[:])

    nc.compile()
    return nc


def get_nc():
    if "nc" not in _nc_cache:
        _nc_cache["nc"] = _build_bass()
    return _nc_cache["nc"]


def _shift(s, d):
    """s: [64, RC, B]; returns s advanced by d blocks along axis 1, zero-filled."""
    out = np.zeros_like(s)
    out[:, : RC - d, :] = s[:, d:, :]
    return out


def pack_inputs(x, weight, bias):
    """Returns list of per-core in_maps (numpy, C-contiguous)."""
    x = np.asarray(x, dtype=np.float32)
    weight = np.asarray(weight, dtype=np.float32)
    bias = np.asarray(bias, dtype=np.float32)

    # padded x: [B, CIN, H+2, W+2]
    xp = np.zeros((B, CIN, H + 2, W + 2), dtype=np.float32)
    xp[:, :, 1:-1, 1:-1] = x

    # weight -> [h, w, ky, kx, cin, cout]
    wt = np.ascontiguousarray(np.transpose(weight, (2, 3, 4, 5, 0, 1)))

    ky0s = np.array([p[0][0] for p in PAIRS])
    kx0s = np.array([p[0][1] for p in PAIRS])
    ky1s = np.array([p[1][0] for p in PAIRS])
    kx1s = np.array([p[1][1] for p in PAIRS])

    in_maps = []
    for c in range(NCORES):
        h0 = c * ROWS
        # x slab rows h0-1 .. h0+ROWS (SLAB_R rows of padded x)
        slab = xp[:, :, h0 : h0 + SLAB_R, :]  # [B, CIN, SLAB_R, SLAB_C]
        s = np.transpose(slab, (1, 2, 3, 0)).reshape(CIN, RC, B)  # [cin, rc, b]
        x1 = np.concatenate([s, _shift(s, 1)], axis=0)
        x34 = np.concatenate([s, _shift(s, 34)], axis=0)

        wh = wt[h0 : h0 + ROWS]  # [ROWS, w, ky, kx, cin, cout]
        top = wh[:, :, ky0s, kx0s]  # [ROWS, w, j, cin, cout]
        bot = wh[:, :, ky1s, kx1s]
        # -> [ROWS, cin, w, j, cout]
        top = np.transpose(top, (0, 3, 1, 2, 4))
        bot = np.transpose(bot, (0, 3, 1, 2, 4))
        wmain = np.concatenate([top, bot], axis=1)  # [ROWS, 128, w, j, cout]
        wtail = np.transpose(wh[:, :, TAIL[0], TAIL[1]], (0, 2, 1, 3))  # [ROWS, cin, w, cout]

        bi = np.transpose(bias[:, h0 : h0 + ROWS, :], (1, 0, 2))  # [ROWS, cout, w]

        in_maps.append(
            {
                "x1": np.ascontiguousarray(x1),
                "x34": np.ascontiguousarray(x34),
                "wmain": np.ascontiguousarray(wmain),
                "wtail": np.ascontiguousarray(wtail),
                "bias": np.ascontiguousarray(bi),
            }
        )
    return in_maps


def unpack_outputs(results):
    """results: list of per-core out_maps with 'out' [ROWS, COUT, WOUT, B]."""
    full = np.concatenate([np.asarray(r["out"]) for r in results], axis=0)
    # [HOUT, COUT, WOUT, B] -> [B, COUT, HOUT, WOUT]
    return np.ascontiguousarray(np.transpose(full, (3, 1, 0, 2)))


def run(in_maps, **kwargs):
    from concourse import bass_utils

    nc = get_nc()
    return bass_utils.run_bass_kernel_spmd(
        nc, in_maps, core_ids=list(range(NCORES)), **kwargs
    )


def kernel(x, weight, bias):
    in_maps = pack_inputs(x, weight, bias)
    res = run(in_maps)
    return unpack_outputs(res.results)


if __name__ == "__main__":
    rng = np.random.default_rng(0)
    x = rng.standard_normal((B, CIN, H, W), dtype=np.float32)
    weight = rng.standard_normal((CIN, COUT, HOUT, WOUT, K, K), dtype=np.float32)
    bias = rng.standard_normal((COUT, HOUT, WOUT), dtype=np.float32)
    y = kernel(x, weight, bias)
    print("out", y.shape, y.dtype)



# revision 2
# speedup vs baseline: 2.1779x; 2.1779x over previous
"""LocallyConnected2d Trainium2 kernel (bf16 W-PAIR scheme).

y[b,o,h,w] = sum_{i,ky,kx} x[b,i,h+ky-1,w+kx-1] * weight[i,o,h,w,ky,kx] + bias[o,h,w]

Shapes: x [64,64,32,32], weight [64,64,32,32,3,3], bias [64,32,32] -> y [64,64,32,32].

Strategy
--------
Spatial sharding over H_out: 8 cores x 4 output rows each (x slab with halo).

Inputs are converted to bf16 on host (rel-err budget 2e-2 >> bf16's ~0.6%);
fp32 matmuls run at 4 cycles/row on TRN2 vs 1 for bf16, and bf16 halves DMA.

Output columns are processed in pairs (A=2t, B=2t+1) sharing one PSUM tile
[128, 64] = [couts of A | couts of B] x batch. Per pair and kernel row ky,
TWO matmuls with K=128, M=128, N=64 cover all six (ky, kx) contributions:

  M1 @ x1[rc(h+ky, 2t)]   : rhs = [x(., 2t) ; x(., 2t+1)]
      cols 0-63  (A): rows 0-63 = wA(ky,0), rows 64-127 = wA(ky,1)
      cols 64-127(B): rows 0-63 = 0,        rows 64-127 = wB(ky,0)
  M2 @ x1[rc(h+ky, 2t+2)] : rhs = [x(., 2t+2) ; x(., 2t+3)]
      cols 0-63  (A): rows 0-63 = wA(ky,2), rows 64-127 = 0
      cols 64-127(B): rows 0-63 = wB(ky,1), rows 64-127 = wB(ky,2)

All stationaries are 128 columns (FWL-eligible in bf16). The zero quadrants
are memset once into two statically allocated weight buffers; the four dense
sub-blocks are DMAed per h-chunk (no zero bytes over HBM).

x1 = [x_slab ; shift1(x_slab)] (shift by one (row,col)-block of 64 batch
values over the flattened padded slab), packed on host in bf16.
"""

import sys

sys.path.insert(0, "/opt/trn_rl_repo")

import numpy as np
import ml_dtypes

BF16 = ml_dtypes.bfloat16

B, CIN, COUT, H, W = 64, 64, 64, 32, 32
K = 3
HOUT, WOUT = 32, 32
NCORES = 8
ROWS = HOUT // NCORES  # output rows per core
SLAB_R = ROWS + 2      # x rows needed per core (halo)
SLAB_C = W + 2         # padded width
RC = SLAB_R * SLAB_C   # flattened (row, col) length
NT = WOUT // 2         # column pairs per row

_nc_cache = {}


def _build_bass():
    import concourse.bass as bass
    import concourse.tile as tile
    from concourse import bacc, mybir

    f32 = mybir.dt.float32
    bf16 = mybir.dt.bfloat16
    nc = bacc.Bacc(None, target_bir_lowering=False)

    x1_d = nc.dram_tensor("x1", (128, RC, B), bf16, kind="ExternalInput")
    # dense stationary sub-blocks, chunked by output row h
    wt_d = nc.dram_tensor("wt", (ROWS, 64, NT, 3, 2, 64), bf16, kind="ExternalInput")
    wb1_d = nc.dram_tensor("wb1", (ROWS, 64, NT, 3, 2, 64), bf16, kind="ExternalInput")
    wb2_d = nc.dram_tensor("wb2", (ROWS, 64, NT, 3, 64), bf16, kind="ExternalInput")
    wt2_d = nc.dram_tensor("wt2", (ROWS, 64, NT, 3, 64), bf16, kind="ExternalInput")
    bias_d = nc.dram_tensor("bias", (128, ROWS, NT), f32, kind="ExternalInput")
    out_d = nc.dram_tensor("out", (ROWS, 128, NT, B), bf16, kind="ExternalOutput")

    with tile.TileContext(nc) as tc:
        with (
            tc.tile_pool(name="xpool", bufs=1) as xpool,
            tc.tile_pool(name="wpool", bufs=1) as wpool,
            tc.tile_pool(name="opool", bufs=2) as opool,
            tc.tile_pool(name="bpool", bufs=1) as bpool,
            tc.tile_pool(name="psum", bufs=8, space=bass.MemorySpace.PSUM) as psum,
        ):
            x1 = xpool.tile([128, RC, B], bf16, tag="x1")
            nc.sync.dma_start(x1[:], x1_d[:])

            bi = bpool.tile([128, ROWS, NT], f32, tag="bias")
            nc.sync.dma_start(bi[:], bias_d[:])

            wk = [
                wpool.tile([128, NT, 3, 2, 128], bf16, tag=f"wk{i}", name=f"wk{i}")
                for i in range(2)
            ]
            for i in range(2):
                nc.any.memset(wk[i][0:64, :, :, 0, 64:128], 0.0)
                nc.any.memset(wk[i][64:128, :, :, 1, 0:64], 0.0)

            for h in range(ROWS):
                wkb = wk[h % 2]
                nc.sync.dma_start(wkb[0:64, :, :, :, 0:64], wt_d[h])
                nc.sync.dma_start(wkb[64:128, :, :, :, 64:128], wb1_d[h])
                nc.sync.dma_start(wkb[64:128, :, :, 0, 0:64], wb2_d[h])
                nc.sync.dma_start(wkb[0:64, :, :, 1, 64:128], wt2_d[h])

                ot = opool.tile([128, NT, B], bf16, tag="out")
                for t in range(NT):
                    ps = psum.tile([128, B], f32, tag="ps")
                    j = 0
                    for ky in range(3):
                        for m in range(2):
                            rc = (h + ky) * SLAB_C + 2 * t + 2 * m
                            nc.tensor.matmul(
                                ps[:],
                                wkb[:, t, ky, m, :],
                                x1[:, rc, :],
                                start=(j == 0),
                                stop=(j == 5),
                            )
                            j += 1
                    nc.any.tensor_scalar_add(ot[:, t, :], ps[:], bi[:, h, t : t + 1])

                nc.sync.dma_start(out_d[h], ot[:])

    nc.compile()
    return nc


def get_nc():
    if "nc" not in _nc_cache:
        _nc_cache["nc"] = _build_bass()
    return _nc_cache["nc"]


def _shift(s, d):
    """s: [64, RC, B]; returns s advanced by d blocks along axis 1, zero-filled."""
    out = np.zeros_like(s)
    out[:, : RC - d, :] = s[:, d:, :]
    return out


def pack_inputs(x, weight, bias):
    """Returns list of per-core in_maps (numpy, C-contiguous)."""
    x = np.asarray(x, dtype=np.float32)
    weight = np.asarray(weight, dtype=np.float32)
    bias = np.asarray(bias, dtype=np.float32)

    # padded x: [B, CIN, H+2, W+2]
    xp = np.zeros((B, CIN, H + 2, W + 2), dtype=np.float32)
    xp[:, :, 1:-1, 1:-1] = x

    # weight -> [h, w, ky, kx, cin, cout] in bf16
    wt_all = np.ascontiguousarray(
        np.transpose(weight, (2, 3, 4, 5, 0, 1))
    ).astype(BF16)

    in_maps = []
    for c in range(NCORES):
        h0 = c * ROWS
        # x slab rows h0-1 .. h0+ROWS (SLAB_R rows of padded x)
        slab = xp[:, :, h0 : h0 + SLAB_R, :]  # [B, CIN, SLAB_R, SLAB_C]
        s = np.transpose(slab, (1, 2, 3, 0)).reshape(CIN, RC, B).astype(BF16)
        x1 = np.concatenate([s, _shift(s, 1)], axis=0)

        wh = wt_all[h0 : h0 + ROWS]  # [ROWS, w, ky, kx, cin, cout]
        # per-pair views: A = even cols (2t), B = odd cols (2t+1)
        wA = wh[:, 0::2]  # [ROWS, NT, ky, kx, cin, cout]
        wB = wh[:, 1::2]

        def pk(a):  # [ROWS, NT, ky, cin, cout] -> [ROWS, cin, NT, ky, cout]
            return np.transpose(a, (0, 3, 1, 2, 4))

        # wt:  top rows (x col 2t / 2t+2), cols 0-63 (loc A): m0=A(ky,0), m1=A(ky,2)
        wt = np.stack([pk(wA[:, :, :, 0]), pk(wA[:, :, :, 2])], axis=4)
        # wb1: bottom rows, cols 64-127 (loc B): m0=B(ky,0), m1=B(ky,2)
        wb1 = np.stack([pk(wB[:, :, :, 0]), pk(wB[:, :, :, 2])], axis=4)
        # wb2: bottom rows, cols 0-63, M1 only: A(ky,1)
        wb2 = pk(wA[:, :, :, 1])
        # wt2: top rows, cols 64-127, M2 only: B(ky,1)
        wt2 = pk(wB[:, :, :, 1])

        # bias image [128, ROWS, NT]: partition s*64+o -> (w=2t+s, cout=o)
        bh = bias[:, h0 : h0 + ROWS, :]  # [cout, ROWS, W]
        bA = bh[:, :, 0::2]  # [cout, ROWS, NT]
        bB = bh[:, :, 1::2]
        bimg = np.concatenate([bA, bB], axis=0)  # [128, ROWS, NT]

        in_maps.append(
            {
                "x1": np.ascontiguousarray(x1),
                "wt": np.ascontiguousarray(wt),
                "wb1": np.ascontiguousarray(wb1),
                "wb2": np.ascontiguousarray(wb2),
                "wt2": np.ascontiguousarray(wt2),
                "bias": np.ascontiguousarray(bimg),
            }
        )
    return in_maps


def unpack_outputs(results):
    """results: list of per-core out_maps with 'out' [ROWS, 128, NT, B] bf16."""
    full = np.concatenate([np.asarray(r["out"]) for r in results], axis=0)
    # [HOUT, 2, COUT, NT, B] -> [B, COUT, HOUT, NT, 2]
    o = full.reshape(HOUT, 2, COUT, NT, B)
    y = np.transpose(o, (4, 2, 0, 3, 1)).reshape(B, COUT, HOUT, WOUT)
    return np.ascontiguousarray(y.astype(np.float32))


def run(in_maps, **kwargs):
    from concourse import bass_utils

    nc = get_nc()
    return bass_utils.run_bass_kernel_spmd(
        nc, in_maps, core_ids=list(range(NCORES)), **kwargs
    )


def kernel(x, weight, bias):
    in_maps = pack_inputs(x, weight, bias)
    res = run(in_maps)
    return unpack_outputs(res.results)


if __name__ == "__main__":
    rng = np.random.default_rng(0)
    x = rng.standard_normal((B, CIN, H, W), dtype=np.float32)
    weight = rng.standard_normal((CIN, COUT, HOUT, WOUT, K, K), dtype=np.float32)
    bias = rng.standard_normal((COUT, HOUT, WOUT), dtype=np.float32)
    y = kernel(x, weight, bias)
    print("out", y.shape, y.dtype)
